# revision 4
# baseline (speedup 1.0000x reference)
"""Self-contained Trainium2 Bass kernel for nn_Attention_395136991961.

Dense multi-head attention (B=8, N=1024, C=1024, H=16, D=64) with RoPE,
full materialized softmax, and output projection.

Sharding: data-parallel over batch B across the 8 NeuronCores (one batch
element per core, weights replicated, no collectives).

Precision: qkv projection and attention logits in float32r (~1.6e-4);
the post-softmax linear path (P, V, O^T, proj weights) in bfloat16.

Key device-side tricks:
  - all accumulation chains interleave TWO PSUM banks (avoids the
    same-bank drain conflict that serializes back-to-back matmuls)
  - S^T for the even/odd head of a pair issued as adjacent K=64 matmuls
    on disjoint PE row groups (execute concurrently)
  - softmax denominators ride along as a ones-column in the V stationary
  - exp ops paired into [128, 1024] to amortize ACT per-op overhead
  - P^T tile pool hoisted so phase B overlaps phase A's tail
"""

import sys

if "/opt/trn_rl_repo" not in sys.path:
    sys.path.insert(0, "/opt/trn_rl_repo")

import numpy as np

import concourse.tile as tile
import concourse.mybir as mybir
from concourse import bacc
from concourse.bass_utils import run_bass_kernel_spmd

F32 = mybir.dt.float32
F32R = mybir.dt.float32r
BF16 = mybir.dt.bfloat16
AF = mybir.ActivationFunctionType
OP = mybir.AluOpType

N_CORES = 8
C = 1024
H = 16
D = 64
HD2 = D // 2  # rotate-half split
SCALE = float(D) ** -0.5

PROFILE = False
LAST_EXEC_NS = None
LAST_TRACE = None
_CACHE = {}


def build(n_tok):
    ntile = n_tok // 128          # token tiles
    mch = 512 if n_tok >= 512 else n_tok
    nmch = n_tok // mch           # m chunks per head
    nct = C // 128                # contraction tiles (8)
    njp = (3 * C) // 1024         # qkv j-chunk pairs (3)

    nc = bacc.Bacc("TRN2", target_bir_lowering=False, debug=False, num_devices=1)

    xT = nc.dram_tensor("xT", [C, n_tok], F32, kind="ExternalInput").ap()
    wT = nc.dram_tensor("wT", [C, 3 * C], F32, kind="ExternalInput").ap()
    pwT = nc.dram_tensor("pwT", [C, C], BF16, kind="ExternalInput").ap()
    pbias = nc.dram_tensor("pbias", [1, C], F32, kind="ExternalInput").ap()
    cosN = nc.dram_tensor("cosN", [n_tok, D], F32, kind="ExternalInput").ap()
    sinA = nc.dram_tensor("sinA", [n_tok, D], F32, kind="ExternalInput").ap()
    identin = nc.dram_tensor("identin", [128, 128], F32, kind="ExternalInput").ap()
    vinit = nc.dram_tensor("vinit", [128, H * (D + 1)], BF16, kind="ExternalInput").ap()
    y = nc.dram_tensor("y", [n_tok, C], F32, kind="ExternalOutput").ap()

    xT_t = xT.rearrange("(t p) n -> p t n", p=128)
    wT_t = wT.rearrange("(t p) j -> p t j", p=128)
    pwT_t = pwT.rearrange("(t p) e -> p t e", p=128)
    cos_t = cosN.rearrange("(t p) d -> p t d", p=128)
    sin_t = sinA.rearrange("(t p) d -> p t d", p=128)

    with tile.TileContext(nc) as tc:
        with (
            tc.tile_pool(name="persist", bufs=1) as pp,
            tc.tile_pool(name="psQ", bufs=2, space="PSUM") as psQ,
            tc.tile_pool(name="psPO", bufs=2, space="PSUM") as psPO,
            tc.tile_pool(name="psBig", bufs=2, space="PSUM") as psBig,
        ):
            # ---------------- persistent tiles ----------------
            qT_sb = pp.tile([128, H // 2, n_tok], F32R, tag="qT")
            kT_sb = pp.tile([128, H // 2, n_tok], F32R, tag="kT")
            v_sb = pp.tile([128, ntile, H, D + 1], BF16, tag="v")
            for t in range(ntile):
                nc.sync.dma_start(
                    v_sb[:, t, :, :].rearrange("p h d -> p (h d)"), vinit[:]
                )
            bias_b = pp.tile([128, C], F32, tag="biasb")

            # ---------------- Phase A: qkv + RoPE + transposes -------------
            with (
                tc.tile_pool(name="xtp", bufs=1) as xtp,
                tc.tile_pool(name="wstream", bufs=3) as wsp,
                tc.tile_pool(name="ropetmp", bufs=2) as rtp,
                tc.tile_pool(name="smalls", bufs=1) as smp,
            ):
                cos_sb = smp.tile([128, ntile, D], F32, tag="cos")
                nc.sync.dma_start(cos_sb[:], cos_t)
                sin_sb = smp.tile([128, ntile, D], F32, tag="sin")
                nc.sync.dma_start(sin_sb[:], sin_t)
                ident = smp.tile([128, 128], F32R, tag="ident")
                nc.sync.dma_start(ident[:], identin[:].bitcast(F32R))
                pb_sb = smp.tile([1, C], F32, tag="pb")
                nc.sync.dma_start(pb_sb[:], pbias[:])
                nc.gpsimd.partition_broadcast(bias_b[:], pb_sb[0:1, :])

                xT_sb = xtp.tile([128, nct, n_tok], F32R, tag="xT")
                nc.sync.dma_start(xT_sb[:], xT_t.bitcast(F32R))

                def rope_and_transpose(pq, jc, t):
                    # pq: [128, 512] psum view holding 8 heads of q or k
                    dstT = qT_sb if jc < 2 else kT_sb
                    half = jc % 2
                    pq3 = pq.rearrange("p (h d) -> p h d", d=D)
                    cos3 = (
                        cos_sb[:, t, :]
                        .rearrange("p (o d) -> p o d", d=D)
                        .to_broadcast([128, 8, D])
                    )
                    sinlo = (
                        sin_sb[:, t, 0:HD2]
                        .rearrange("p (o d) -> p o d", d=HD2)
                        .to_broadcast([128, 8, HD2])
                    )
                    sinhi = (
                        sin_sb[:, t, HD2:D]
                        .rearrange("p (o d) -> p o d", d=HD2)
                        .to_broadcast([128, 8, HD2])
                    )
                    tmp = rtp.tile([128, 512], F32, tag="ropet")
                    tmp3 = tmp[:].rearrange("p (h d) -> p h d", d=D)
                    nc.vector.tensor_tensor(
                        out=tmp3[:, :, 0:HD2], in0=pq3[:, :, HD2:D],
                        in1=sinlo, op=OP.mult,
                    )
                    nc.vector.tensor_tensor(
                        out=tmp3[:, :, HD2:D], in0=pq3[:, :, 0:HD2],
                        in1=sinhi, op=OP.mult,
                    )
                    u = rtp.tile([128, 512], F32, tag="ropeu")
                    nc.vector.tensor_tensor(
                        out=u[:].rearrange("p (h d) -> p h d", d=D),
                        in0=pq3, in1=cos3, op=OP.mult,
                    )
                    qh = rtp.tile([128, 512], F32R, tag="ropeq")
                    nc.vector.tensor_tensor(
                        out=qh[:], in0=u[:], in1=tmp[:], op=OP.add
                    )
                    for jb in range(4):
                        pt = psBig.tile([128, 128], F32R, tag="big")
                        nc.tensor.transpose(
                            pt[:], qh[:, jb * 128 : (jb + 1) * 128], ident[:]
                        )
                        nc.vector.tensor_copy(
                            dstT[:, half * 4 + jb, t * 128 : (t + 1) * 128],
                            pt[:],
                        )

                for jc in range(2 * njp):  # 6 j-chunks
                    wchunk = wsp.tile([128, nct, 512], F32R, tag="w")
                    nc.sync.dma_start(
                        wchunk[:],
                        wT_t[:, :, jc * 512 : (jc + 1) * 512].bitcast(F32R),
                    )
                    for t in range(ntile):
                        pq = psQ.tile([128, 512], F32, tag="pq")
                        for ct in range(nct):
                            nc.tensor.matmul(
                                pq[:],
                                xT_sb[:, ct, t * 128 : (t + 1) * 128],
                                wchunk[:, ct, :],
                                start=(ct == 0),
                                stop=(ct == nct - 1),
                            )
                        if jc < 4:
                            rope_and_transpose(pq[:], jc, t)
                        else:
                            hb = (jc - 4) * 8
                            nc.vector.tensor_copy(
                                v_sb[:, t, hb : hb + 8, 0:D],
                                pq[:].rearrange("p (h d) -> p h d", d=D),
                            )

            # ------------- Phase B + C (oT spans both) ----------------------
            with tc.tile_pool(name="otp", bufs=1) as otp:
                oT_sb = otp.tile([128, nct, n_tok], BF16, tag="oT")

                with (
                    tc.tile_pool(name="ptpool", bufs=1) as ptp,
                    tc.tile_pool(name="nrm", bufs=2) as nrm,
                    tc.tile_pool(name="pwp", bufs=1) as pwp,
                    tc.tile_pool(name="ypool", bufs=2) as yp,
                ):
                    pwc = pwp.tile([128, nct, 2, 512], BF16, tag="pw")
                    nc.sync.dma_start(
                        pwc[:],
                        pwT_t.rearrange("p t (a e) -> p t a e", a=2),
                    )
                    for mc in range(nmch):
                        ms = mc * mch
                        for jt in range(H // 2):
                            pTe = ptp.tile([128, ntile, mch], BF16, tag="pTe")
                            pTo = ptp.tile([128, ntile, mch], BF16, tag="pTo")
                            for tp_ in range(ntile // 2):
                                t0 = 2 * tp_
                                pse = psBig.tile([128, 2, mch], F32, tag="big")
                                pso = psBig.tile([128, 2, mch], F32, tag="big")
                                for i in range(2):
                                    t = t0 + i
                                    nc.tensor.matmul(
                                        pse[:, i, :],
                                        kT_sb[0:64, jt, t * 128 : (t + 1) * 128],
                                        qT_sb[0:64, jt, ms : ms + mch],
                                        start=True,
                                        stop=True,
                                    )
                                    nc.tensor.matmul(
                                        pso[:, i, :],
                                        kT_sb[64:128, jt, t * 128 : (t + 1) * 128],
                                        qT_sb[64:128, jt, ms : ms + mch],
                                        start=True,
                                        stop=True,
                                    )
                                nc.scalar.activation(
                                    pTe[:, t0 : t0 + 2, :].rearrange(
                                        "p a m -> p (a m)"
                                    ),
                                    pse[:].rearrange("p a m -> p (a m)"),
                                    AF.Exp,
                                    scale=SCALE,
                                )
                                nc.scalar.activation(
                                    pTo[:, t0 : t0 + 2, :].rearrange(
                                        "p a m -> p (a m)"
                                    ),
                                    pso[:].rearrange("p a m -> p (a m)"),
                                    AF.Exp,
                                    scale=SCALE,
                                )
                            poE = psPO.tile([65, mch], F32, tag="po")
                            poO = psPO.tile([65, mch], F32, tag="po")
                            for t in range(ntile):
                                nc.tensor.matmul(
                                    poE[:],
                                    v_sb[:, t, 2 * jt, :],
                                    pTe[:, t, :],
                                    start=(t == 0),
                                    stop=(t == ntile - 1),
                                )
                            for t in range(ntile):
                                nc.tensor.matmul(
                                    poO[:],
                                    v_sb[:, t, 2 * jt + 1, :],
                                    pTo[:, t, :],
                                    start=(t == 0),
                                    stop=(t == ntile - 1),
                                )
                            for par, po in ((0, poE), (1, poO)):
                                ssb = nrm.tile([128, mch], F32, tag="ssb")
                                nc.scalar.copy(ssb[64:65, :], po[64:65, :])
                                s0 = nrm.tile([1, mch], F32, tag="s0")
                                nc.sync.dma_start(s0[:], ssb[64:65, :])
                                rs0 = nrm.tile([1, mch], F32, tag="rs0")
                                nc.vector.reciprocal_approx_fast(
                                    out=rs0[:], in_=s0[:]
                                )
                                rb_sb = nrm.tile([64, mch], F32, tag="rb")
                                nc.gpsimd.partition_broadcast(
                                    rb_sb[:], rs0[0:1, :]
                                )
                                tmpo = nrm.tile([64, mch], BF16, tag="tmpo")
                                nc.vector.tensor_tensor(
                                    out=tmpo[:],
                                    in0=po[0:64, :],
                                    in1=rb_sb[:],
                                    op=OP.mult,
                                )
                                nc.sync.dma_start(
                                    oT_sb[
                                        par * 64 : par * 64 + 64,
                                        jt,
                                        ms : ms + mch,
                                    ],
                                    tmpo[:],
                                )

                        # proj for this m-chunk's tokens (all heads done)
                        for t in range(
                            mc * (mch // 128), (mc + 1) * (mch // 128)
                        ):
                            for ec in range(2):
                                py = psQ.tile([128, 512], F32, tag="pq")
                                for ft in range(nct):
                                    nc.tensor.matmul(
                                        py[:],
                                        oT_sb[:, ft, t * 128 : (t + 1) * 128],
                                        pwc[:, ft, ec, :],
                                        start=(ft == 0),
                                        stop=(ft == nct - 1),
                                    )
                                ysb = yp.tile([128, 512], F32, tag="y")
                                nc.vector.tensor_tensor(
                                    out=ysb[:],
                                    in0=py[:],
                                    in1=bias_b[:, ec * 512 : (ec + 1) * 512],
                                    op=OP.add,
                                )
                                nc.sync.dma_start(
                                    y[
                                        t * 128 : (t + 1) * 128,
                                        ec * 512 : (ec + 1) * 512,
                                    ],
                                    ysb[:],
                                )

    nc.compile()
    return nc


def _host_inputs(x, rope_freqs, qkv_w, proj_w, proj_b):
    import ml_dtypes

    x = np.asarray(x, dtype=np.float32)
    rope_freqs = np.asarray(rope_freqs, dtype=np.float32)
    qkv_w = np.asarray(qkv_w, dtype=np.float32)
    proj_w = np.asarray(proj_w, dtype=np.float32)
    proj_b = np.asarray(proj_b, dtype=np.float32)

    B, n_tok, _ = x.shape
    wTh = np.ascontiguousarray(qkv_w.T)
    pwTh = np.ascontiguousarray(proj_w.T).astype(ml_dtypes.bfloat16)
    freqs = rope_freqs[0, :, 0, :]  # [N, D]
    cosh = np.cos(freqs).astype(np.float32)
    sinh = np.sin(freqs).astype(np.float32)
    sinAh = np.concatenate([-sinh[:, :HD2], sinh[:, HD2:]], axis=1)
    sinAh = np.ascontiguousarray(sinAh)
    identh = np.eye(128, dtype=np.float32)
    vinith = np.zeros((128, H, D + 1), np.float32)
    vinith[:, :, D] = 1.0
    vinith = vinith.reshape(128, H * (D + 1)).astype(ml_dtypes.bfloat16)
    pbh = np.ascontiguousarray(proj_b.reshape(1, C))

    in_maps = []
    for b in range(B):
        in_maps.append(
            {
                "xT": np.ascontiguousarray(x[b].T),
                "wT": wTh,
                "pwT": pwTh,
                "pbias": pbh,
                "cosN": cosh,
                "sinA": sinAh,
                "identin": identh,
                "vinit": vinith,
            }
        )
    return in_maps, n_tok


def kernel(x, rope_freqs, qkv_w, proj_w, proj_b):
    global LAST_EXEC_NS, LAST_TRACE
    in_maps, n_tok = _host_inputs(x, rope_freqs, qkv_w, proj_w, proj_b)
    key = ("nc", n_tok)
    if key not in _CACHE:
        _CACHE[key] = build(n_tok)
    nc = _CACHE[key]

    trace = False
    if PROFILE:
        try:
            import profshim

            profshim.install()
            trace = True
        except Exception:
            trace = False

    res = run_bass_kernel_spmd(
        nc, in_maps, list(range(len(in_maps))), trace=trace
    )
    LAST_EXEC_NS = res.exec_time_ns
    LAST_TRACE = res.instructions_and_trace
    out = np.stack([res.results[b]["y"] for b in range(len(in_maps))], axis=0)
    return out

